# revision 37
# baseline (speedup 1.0000x reference)
"""RelationNetwork (B=32, N=64, D=128) Bass/Tile kernel for 8 TRN2 NeuronCores.

Strategy: pure data-parallel over the batch (4 batch elements/core).
  - BN (eval mode) is folded into the Linear weights/biases on the host.
  - Layer G1 is separable: h1[i,j] = relu(A'_i + R_j) with A = x @ W0L'^T + b1,
    R = x @ W0R'^T computed once per core (N rows instead of N^2).
  - Layers G2/G3 are dense matmuls over the 4096-pair axis, processed in
    512-pair chunks held entirely on-chip (feature-major layout [T, pairs]).
  - The pair-sum is fused into the G3 PSUM evacuations (ACT accum_out on one
    half; a DVE max-trick accumulate on the other, bias-corrected in fb1).
  - The tiny F head runs on-device on the pooled [T, 4] activations.
Matmuls for G2/G3 use float32r (full PE rate at free-dim 512, fp32 storage).
All tiles are statically allocated (manual slot rotation, no pool recycling):
on this execution environment, tile-pool alloc/release machinery costs
~100+us per event, dominating everything else.
"""
import numpy as np

import concourse.bass as bass
import concourse.tile as tile
from concourse import bacc, mybir
from concourse.bass_utils import run_bass_kernel_spmd

N_CORES = 8
B, N, D = 32, 64, 128
T = 2 * D            # 256
OUT = 128
BC = B // N_CORES    # batch elements per core = 4
IC = 8               # i-values per pair chunk  -> chunk = IC*N = 512 pairs
NCHUNK_B = N // IC   # chunks per batch element = 8
NCHUNK = BC * NCHUNK_B  # 32 chunks per core
CH = IC * N          # 512
EPS = 1e-5
NSLOT = 8            # static slot-ring depth for h1/h2 buffers

F32 = mybir.dt.float32
F32R = mybir.dt.float32r
AX = mybir.AxisListType
ALU = mybir.AluOpType
ACTF = mybir.ActivationFunctionType

_CACHE: dict = {}
TRACE = False
LAST = None  # BassKernelResults of the most recent run (for profiling)


def _build(repeat=1):
    nc = bacc.Bacc("TRN2", target_bir_lowering=False, debug=False,
                   enable_asserts=False, num_devices=N_CORES)

    def din(name, shape, dt=F32):
        return nc.dram_tensor(name, list(shape), dt, kind="ExternalInput").ap()

    xt_d = din("xt", (128, BC * N), F32R)          # x^T per core: [d, (b,i)]
    w0l_d = din("w0l", (128, T), F32R)             # (s0*W0[:, :D])^T  [d, u]
    w0r_d = din("w0r", (128, T), F32R)
    w2_d = din("w2", (2, 2, 128, 128), F32R)       # [k, u2, t_in, u_in]
    w3_d = din("w3", (2, 2, 128, 128), F32R)
    b1_d = din("b1", (128, 2))               # col k = bias chunk k
    b2_d = din("b2", (128, 2))
    b3_d = din("b3", (128, 2))               # col0 = -b3[:128], col1 = +b3[128:]
    fw1_d = din("fw1", (2, 2, 128, 128))
    fw2_d = din("fw2", (2, 2, 128, 128))
    fb1_d = din("fb1", (128, 2))
    fb2_d = din("fb2", (128, 2))
    fow_d = din("fow", (2, 128, 128))        # fo_W^T tiled [k, t_in, o]
    fob_d = din("fob", (BC, OUT))            # fo_b replicated across BC rows
    out_d = nc.dram_tensor("out", [BC, OUT], F32, kind="ExternalOutput").ap()

    with tile.TileContext(nc) as tc:
        with (
            tc.tile_pool(name="wpool", bufs=1) as wp,
            tc.tile_pool(name="pspool", bufs=1, space="PSUM") as psp,
        ):
            # ---- persistent loads ----
            def load(dram_ap, shape, tag, dt=F32):
                t = wp.tile(list(shape), dt, tag=tag, name=tag)
                nc.sync.dma_start(t[:], dram_ap)
                return t

            xt = load(xt_d[:], (128, BC * N), "xt", F32R)
            w0l = load(w0l_d[:], (128, T), "w0l", F32R)
            w0r = load(w0r_d[:], (128, T), "w0r", F32R)
            b1t = load(b1_d[:], (128, 2), "b1t")
            b2t = load(b2_d[:], (128, 2), "b2t")
            b3t = load(b3_d[:], (128, 2), "b3t")
            w2 = [[load(w2_d[k, u2], (128, 128), f"w2_{k}{u2}", F32R)
                   for u2 in range(2)] for k in range(2)]
            w3 = [[load(w3_d[k, u2], (128, 128), f"w3_{k}{u2}", F32R)
                   for u2 in range(2)] for k in range(2)]
            fw1 = [[load(fw1_d[k, u2], (128, 128), f"fw1_{k}{u2}")
                    for u2 in range(2)] for k in range(2)]
            fw2 = [[load(fw2_d[k, u2], (128, 128), f"fw2_{k}{u2}")
                    for u2 in range(2)] for k in range(2)]
            fow = [load(fow_d[k], (128, 128), f"fow_{k}") for k in range(2)]
            fb1t = load(fb1_d[:], (128, 2), "fb1t")
            fb2t = load(fb2_d[:], (128, 2), "fb2t")
            fobt = load(fob_d[:], (BC, OUT), "fobt")

            # static compute buffers (no recycling)
            at = wp.tile([128, 2 * BC * N], F32, tag="at", name="at")
            rt = wp.tile([128, 2 * BC * N], F32, tag="rt", name="rt")
            h1pre = [wp.tile([128, 2 * CH], F32, tag=f"h1p{j}", name=f"h1p{j}")
                     for j in range(NSLOT)]
            h1 = [wp.tile([128, 2 * CH], F32R, tag=f"h1_{j}", name=f"h1_{j}")
                  for j in range(NSLOT)]
            h2 = [wp.tile([128, 2 * CH], F32R, tag=f"h2_{j}", name=f"h2_{j}")
                  for j in range(NSLOT)]
            acc = [wp.tile([128, NCHUNK], F32, tag=f"acc{u2}", name=f"acc{u2}")
                   for u2 in range(2)]
            pooled = [wp.tile([128, BC], F32, tag=f"pl{u2}", name=f"pl{u2}")
                      for u2 in range(2)]
            hf1 = [wp.tile([128, BC], F32, tag=f"hf1{u2}", name=f"hf1{u2}")
                   for u2 in range(2)]
            hf2 = [wp.tile([128, BC], F32, tag=f"hf2{u2}", name=f"hf2{u2}")
                   for u2 in range(2)]
            outt = wp.tile([BC, OUT], F32, tag="outt", name="outt")
            # 8 static PSUM banks, rotated manually
            pb = [psp.tile([128, CH], F32, tag=f"pb{j}", name=f"pb{j}")
                  for j in range(8)]

            for _ in range(repeat):
                # ---- G1 setup: A' = x @ W0L'^T + b1 ; R = x @ W0R'^T ----
                # at/rt cols = u2*256 + (b*64 + i)
                for u2 in range(2):
                    psa = pb[u2]
                    nc.tensor.matmul(psa[:, :BC * N],
                                     w0l[:, u2 * 128:(u2 + 1) * 128],
                                     xt[:], start=True, stop=True)
                    nc.scalar.activation(at[:, u2 * BC * N:(u2 + 1) * BC * N],
                                         psa[:, :BC * N], ACTF.Identity,
                                         bias=b1t[:, u2:u2 + 1])
                    psr = pb[2 + u2]
                    nc.tensor.matmul(psr[:, :BC * N],
                                     w0r[:, u2 * 128:(u2 + 1) * 128],
                                     xt[:], start=True, stop=True)
                    nc.vector.tensor_copy(rt[:, u2 * BC * N:(u2 + 1) * BC * N],
                                          psr[:, :BC * N])

                # ---- main loop over 512-pair chunks ----
                for b in range(BC):
                    for ic in range(NCHUNK_B):
                        c = b * NCHUNK_B + ic
                        s = c % NSLOT
                        ps2 = [pb[(4 * c) % 8], pb[(4 * c + 1) % 8]]
                        ps3 = [pb[(4 * c + 2) % 8], pb[(4 * c + 3) % 8]]
                        # h1[:, k*512 + i*64 + j] = relu(A'[u,i] + R'[u,j])
                        a_sl = (at[:].rearrange("p (v c) -> p v c", v=2)
                                [:, :, b * N + ic * IC: b * N + (ic + 1) * IC])
                        a_b = a_sl.unsqueeze(3).broadcast_to((128, 2, IC, N))
                        r_sl = (rt[:].rearrange("p (v c) -> p v c", v=2)
                                [:, :, b * N:(b + 1) * N])
                        r_b = r_sl.unsqueeze(2).broadcast_to((128, 2, IC, N))
                        h1v = h1pre[s][:].rearrange("p (v a b) -> p v a b",
                                                    v=2, b=N)
                        nc.vector.tensor_tensor(h1v, a_b, r_b, op=ALU.add)
                        nc.gpsimd.tensor_scalar_max(h1[s][:], h1pre[s][:], 0.0)

                        # G2: ps2[u2] = sum_k w2[k][u2]^T @ h1[k]
                        for u2 in range(2):
                            for k in range(2):
                                nc.tensor.matmul(ps2[u2][:], w2[k][u2][:],
                                                 h1[s][:, k * CH:(k + 1) * CH],
                                                 start=(k == 0), stop=(k == 1))
                        for u2 in range(2):
                            nc.scalar.activation(
                                h2[s][:, u2 * CH:(u2 + 1) * CH], ps2[u2][:],
                                ACTF.Relu, bias=b2t[:, u2:u2 + 1])

                        # G3 + fused pair-sum into acc column
                        for u2 in range(2):
                            for k in range(2):
                                nc.tensor.matmul(ps3[u2][:], w3[k][u2][:],
                                                 h2[s][:, k * CH:(k + 1) * CH],
                                                 start=(k == 0), stop=(k == 1))
                        # u2=0 on DVE: sum max(x,-b3) (fb1 corrected on host)
                        nc.vector.tensor_scalar(ps3[0][:], ps3[0][:],
                                                b3t[:, 0:1], None,
                                                op0=ALU.max, op1=ALU.add,
                                                accum_out=acc[0][:, c:c + 1])
                        # u2=1 on ACT: fused relu + accumulate
                        nc.scalar.activation(ps3[1][:], ps3[1][:], ACTF.Relu,
                                             bias=b3t[:, 1:2],
                                             accum_out=acc[1][:, c:c + 1])

                # ---- pooled sum per batch element: [128, BC] per u-chunk ----
                for u2 in range(2):
                    accv = acc[u2][:].rearrange("p (b c) -> p b c", c=NCHUNK_B)
                    nc.vector.reduce_sum(pooled[u2][:], accv, axis=AX.X)

                # ---- F head (full fp32 matmuls; tiny) ----
                for u2 in range(2):
                    psf = pb[u2]
                    for k in range(2):
                        nc.tensor.matmul(psf[:, :BC], fw1[k][u2][:],
                                         pooled[k][:],
                                         start=(k == 0), stop=(k == 1))
                    nc.scalar.activation(hf1[u2][:], psf[:, :BC], ACTF.Relu,
                                         bias=fb1t[:, u2:u2 + 1])
                for u2 in range(2):
                    psf = pb[2 + u2]
                    for k in range(2):
                        nc.tensor.matmul(psf[:, :BC], fw2[k][u2][:],
                                         hf1[k][:],
                                         start=(k == 0), stop=(k == 1))
                    nc.scalar.activation(hf2[u2][:], psf[:, :BC], ACTF.Relu,
                                         bias=fb2t[:, u2:u2 + 1])
                # out[b, o] = sum_t hf2^T[t, b] * foW^T[t, o] + fo_b
                pso = pb[4]
                for k in range(2):
                    nc.tensor.matmul(pso[:BC, :OUT], hf2[k][:], fow[k][:],
                                     start=(k == 0), stop=(k == 1))
                nc.vector.tensor_tensor(outt[:], pso[:BC, :OUT], fobt[:],
                                        op=ALU.add)
                nc.sync.dma_start(out_d[:], outt[:])

    nc.compile()
    return nc


def _prep(inputs):
    """Host-side: fold BN into weights, transpose to device layouts."""
    g_W, g_b = inputs["g_W"], inputs["g_b"]
    f_W, f_b = inputs["f_W"], inputs["f_b"]

    def bn_fold(W, bvec, gamma, beta, mean, var):
        s = gamma / np.sqrt(var + EPS)
        We = s[:, None] * W
        be = s * bvec + beta - mean * s
        return We.astype(np.float32), be.astype(np.float32)

    def cols2(v):  # [T] -> [128, 2]
        return np.stack([v[:128], v[128:]], axis=1).astype(np.float32)

    def tile4(Wt):  # [T, T] (t, u) -> [2, 2, 128, 128] = [k, u2, t_in, u_in]
        return np.ascontiguousarray(
            Wt.reshape(2, 128, 2, 128).transpose(0, 2, 1, 3)).astype(np.float32)

    W0e, b1e = bn_fold(g_W[0], g_b[0], inputs["g_gamma"][0], inputs["g_beta"][0],
                       inputs["g_mean"][0], inputs["g_var"][0])
    W2e, b2e = bn_fold(g_W[1], g_b[1], inputs["g_gamma"][1], inputs["g_beta"][1],
                       inputs["g_mean"][1], inputs["g_var"][1])
    W3e, b3e = bn_fold(g_W[2], g_b[2], inputs["g_gamma"][2], inputs["g_beta"][2],
                       inputs["g_mean"][2], inputs["g_var"][2])
    F1e, fb1e = bn_fold(f_W[0], f_b[0], inputs["f_gamma"][0], inputs["f_beta"][0],
                        inputs["f_mean"][0], inputs["f_var"][0])
    F2e, fb2e = bn_fold(f_W[1], f_b[1], inputs["f_gamma"][1], inputs["f_beta"][1],
                        inputs["f_mean"][1], inputs["f_var"][1])
    # the DVE max-trick pair-sum for the u2=0 half omits +b3 per element;
    # compensate in the F1 bias: pooled_true = pooled_raw + N*N*b3 (half 0)
    corr = np.zeros(T, dtype=np.float64)
    corr[:128] = float(N * N) * b3e[:128].astype(np.float64)
    fb1e = (fb1e.astype(np.float64) + F1e.astype(np.float64) @ corr).astype(np.float32)

    shared = {
        "w0l": np.ascontiguousarray(W0e[:, :D].T),
        "w0r": np.ascontiguousarray(W0e[:, D:].T),
        "b1": cols2(b1e),
        "w2": tile4(W2e.T), "b2": cols2(b2e),
        "w3": tile4(W3e.T),
        "b3": np.stack([-b3e[:128], b3e[128:]], axis=1).astype(np.float32),
        "fw1": tile4(F1e.T), "fb1": cols2(fb1e),
        "fw2": tile4(F2e.T), "fb2": cols2(fb2e),
        "fow": np.ascontiguousarray(
            inputs["fo_W"].T.reshape(2, 128, OUT)).astype(np.float32),
        "fob": np.tile(inputs["fo_b"].reshape(1, OUT), (BC, 1)).astype(np.float32),
    }
    x = inputs["x"]
    in_maps = []
    for c in range(N_CORES):
        xc = x[c * BC:(c + 1) * BC]  # [BC, N, D]
        xt = np.ascontiguousarray(xc.transpose(2, 0, 1).reshape(D, BC * N))
        in_maps.append({"xt": xt.astype(np.float32), **shared})
    return in_maps


def kernel(**inputs):
    global LAST
    inputs = {k: np.asarray(v, dtype=np.float32) for k, v in inputs.items()}
    if "nc" not in _CACHE:
        _CACHE["nc"] = _build()
    nc = _CACHE["nc"]
    in_maps = _prep(inputs)
    res = run_bass_kernel_spmd(nc, in_maps, core_ids=list(range(N_CORES)),
                               trace=TRACE)
    LAST = res
    out = np.concatenate([res.results[c]["out"] for c in range(N_CORES)], axis=0)
    return out.astype(np.float32)


# revision 41
# speedup vs baseline: 1.0210x; 1.0210x over previous
"""RelationNetwork (B=32, N=64, D=128) Bass/Tile kernel for 8 TRN2 NeuronCores.

Strategy: pure data-parallel over the batch (4 batch elements/core).
  - BN (eval mode) is folded into the Linear weights/biases on the host.
  - Layer G1 is separable: h1[i,j] = relu(A'_i + R_j) with A = x @ W0L'^T + b1,
    R = x @ W0R'^T computed once per core (N rows instead of N^2).
  - Layers G2/G3 are dense matmuls over the 4096-pair axis, processed in
    512-pair chunks held entirely on-chip (feature-major layout [T, pairs]).
  - The pair-sum is fused into the G3 PSUM evacuations (ACT accum_out on one
    half; a DVE max-trick accumulate on the other, bias-corrected in fb1).
  - The tiny F head runs on-device on the pooled [T, 4] activations.
Matmuls for G2/G3 use float32r (full PE rate at free-dim 512, fp32 storage).
All tiles are statically allocated (manual slot rotation, no pool recycling):
on this execution environment, tile-pool alloc/release machinery costs
~100+us per event, dominating everything else.
"""
import numpy as np

import concourse.bass as bass
import concourse.tile as tile
from concourse import bacc, mybir
from concourse.bass_utils import run_bass_kernel_spmd

N_CORES = 8
B, N, D = 32, 64, 128
T = 2 * D            # 256
OUT = 128
BC = B // N_CORES    # batch elements per core = 4
IC = 8               # i-values per pair chunk  -> chunk = IC*N = 512 pairs
NCHUNK_B = N // IC   # chunks per batch element = 8
NCHUNK = BC * NCHUNK_B  # 32 chunks per core
CH = IC * N          # 512
EPS = 1e-5
NSLOT = 8            # static slot-ring depth for h1/h2 buffers

F32 = mybir.dt.float32
F32R = mybir.dt.float32r
F16 = mybir.dt.float16
AX = mybir.AxisListType
ALU = mybir.AluOpType
ACTF = mybir.ActivationFunctionType

_CACHE: dict = {}
TRACE = False
LAST = None  # BassKernelResults of the most recent run (for profiling)


def _build(repeat=1):
    nc = bacc.Bacc("TRN2", target_bir_lowering=False, debug=False,
                   enable_asserts=False, num_devices=N_CORES)

    def din(name, shape, dt=F32):
        return nc.dram_tensor(name, list(shape), dt, kind="ExternalInput").ap()

    xt_d = din("xt", (128, BC * N), F16)           # x^T per core: [d, (b,i)]
    w0l_d = din("w0l", (128, T), F16)              # (s0*W0[:, :D])^T  [d, u]
    w0r_d = din("w0r", (128, T), F16)
    w2_d = din("w2", (2, 2, 128, 128), F32R)       # [k, u2, t_in, u_in]
    w3_d = din("w3", (2, 2, 128, 128), F32R)
    b1_d = din("b1", (128, 2))               # col k = bias chunk k
    b2_d = din("b2", (128, 2))
    b3_d = din("b3", (128, 2))               # col0 = -b3[:128], col1 = +b3[128:]
    fw1_d = din("fw1", (2, 2, 128, 128))
    fw2_d = din("fw2", (2, 2, 128, 128))
    fb1_d = din("fb1", (128, 2))
    fb2_d = din("fb2", (128, 2))
    fow_d = din("fow", (2, 128, 128))        # fo_W^T tiled [k, t_in, o]
    fob_d = din("fob", (BC, OUT))            # fo_b replicated across BC rows
    out_d = nc.dram_tensor("out", [BC, OUT], F32, kind="ExternalOutput").ap()

    with tile.TileContext(nc) as tc:
        with (
            tc.tile_pool(name="wpool", bufs=1) as wp,
            tc.tile_pool(name="pspool", bufs=1, space="PSUM") as psp,
        ):
            # ---- persistent loads ----
            def load(dram_ap, shape, tag, dt=F32):
                t = wp.tile(list(shape), dt, tag=tag, name=tag)
                nc.sync.dma_start(t[:], dram_ap)
                return t

            # warm the ACT function table during the DMA window (the
            # implicit LoadActFuncSet otherwise lands on the startup
            # critical chain, ~1.3us)
            dummy = wp.tile([1, 2], F32, tag="dummy", name="dummy")
            nc.gpsimd.memset(dummy[:], 0.0)
            nc.scalar.activation(dummy[:, 0:1], dummy[:, 0:1], ACTF.Relu,
                                 bias=0.0)

            xt = load(xt_d[:], (128, BC * N), "xt", F16)
            w0l = load(w0l_d[:], (128, T), "w0l", F16)
            w0r = load(w0r_d[:], (128, T), "w0r", F16)
            b1t = load(b1_d[:], (128, 2), "b1t")
            b2t = load(b2_d[:], (128, 2), "b2t")
            b3t = load(b3_d[:], (128, 2), "b3t")
            w2 = [[load(w2_d[k, u2], (128, 128), f"w2_{k}{u2}", F32R)
                   for u2 in range(2)] for k in range(2)]
            w3 = [[load(w3_d[k, u2], (128, 128), f"w3_{k}{u2}", F32R)
                   for u2 in range(2)] for k in range(2)]
            fw1 = [[load(fw1_d[k, u2], (128, 128), f"fw1_{k}{u2}")
                    for u2 in range(2)] for k in range(2)]
            fw2 = [[load(fw2_d[k, u2], (128, 128), f"fw2_{k}{u2}")
                    for u2 in range(2)] for k in range(2)]
            fow = [load(fow_d[k], (128, 128), f"fow_{k}") for k in range(2)]
            fb1t = load(fb1_d[:], (128, 2), "fb1t")
            fb2t = load(fb2_d[:], (128, 2), "fb2t")
            fobt = load(fob_d[:], (BC, OUT), "fobt")

            # static compute buffers (no recycling)
            at = wp.tile([128, 2 * BC * N], F32, tag="at", name="at")
            rt = wp.tile([128, 2 * BC * N], F32, tag="rt", name="rt")
            h1pre = [wp.tile([128, 2 * CH], F32, tag=f"h1p{j}", name=f"h1p{j}")
                     for j in range(NSLOT)]
            h1 = [wp.tile([128, 2 * CH], F32R, tag=f"h1_{j}", name=f"h1_{j}")
                  for j in range(NSLOT)]
            h2 = [wp.tile([128, 2 * CH], F32R, tag=f"h2_{j}", name=f"h2_{j}")
                  for j in range(NSLOT)]
            acc = [wp.tile([128, NCHUNK], F32, tag=f"acc{u2}", name=f"acc{u2}")
                   for u2 in range(2)]
            pooled = [wp.tile([128, BC], F32, tag=f"pl{u2}", name=f"pl{u2}")
                      for u2 in range(2)]
            hf1 = [wp.tile([128, BC], F32, tag=f"hf1{u2}", name=f"hf1{u2}")
                   for u2 in range(2)]
            hf2 = [wp.tile([128, BC], F32, tag=f"hf2{u2}", name=f"hf2{u2}")
                   for u2 in range(2)]
            outt = wp.tile([BC, OUT], F32, tag="outt", name="outt")
            # 8 static PSUM banks, rotated manually
            pb = [psp.tile([128, CH], F32, tag=f"pb{j}", name=f"pb{j}")
                  for j in range(8)]

            for _ in range(repeat):
                # ---- G1 setup: A' = x @ W0L'^T + b1 ; R = x @ W0R'^T ----
                # at/rt cols = u2*256 + (b*64 + i)
                for u2 in range(2):
                    psa = pb[u2]
                    nc.tensor.matmul(psa[:, :BC * N],
                                     w0l[:, u2 * 128:(u2 + 1) * 128],
                                     xt[:], start=True, stop=True)
                    nc.scalar.activation(at[:, u2 * BC * N:(u2 + 1) * BC * N],
                                         psa[:, :BC * N], ACTF.Identity,
                                         bias=b1t[:, u2:u2 + 1])
                    psr = pb[2 + u2]
                    nc.tensor.matmul(psr[:, :BC * N],
                                     w0r[:, u2 * 128:(u2 + 1) * 128],
                                     xt[:], start=True, stop=True)
                    nc.vector.tensor_copy(rt[:, u2 * BC * N:(u2 + 1) * BC * N],
                                          psr[:, :BC * N])

                # ---- main loop over 512-pair chunks ----
                for b in range(BC):
                    for ic in range(NCHUNK_B):
                        c = b * NCHUNK_B + ic
                        s = c % NSLOT
                        ps2 = [pb[(4 * c) % 8], pb[(4 * c + 1) % 8]]
                        ps3 = [pb[(4 * c + 2) % 8], pb[(4 * c + 3) % 8]]
                        # h1[:, k*512 + i*64 + j] = relu(A'[u,i] + R'[u,j])
                        a_sl = (at[:].rearrange("p (v c) -> p v c", v=2)
                                [:, :, b * N + ic * IC: b * N + (ic + 1) * IC])
                        a_b = a_sl.unsqueeze(3).broadcast_to((128, 2, IC, N))
                        r_sl = (rt[:].rearrange("p (v c) -> p v c", v=2)
                                [:, :, b * N:(b + 1) * N])
                        r_b = r_sl.unsqueeze(2).broadcast_to((128, 2, IC, N))
                        h1v = h1pre[s][:].rearrange("p (v a b) -> p v a b",
                                                    v=2, b=N)
                        nc.vector.tensor_tensor(h1v, a_b, r_b, op=ALU.add)
                        nc.gpsimd.tensor_scalar_max(h1[s][:], h1pre[s][:], 0.0)

                        # G2: ps2[u2] = sum_k w2[k][u2]^T @ h1[k]
                        for u2 in range(2):
                            for k in range(2):
                                nc.tensor.matmul(ps2[u2][:], w2[k][u2][:],
                                                 h1[s][:, k * CH:(k + 1) * CH],
                                                 start=(k == 0), stop=(k == 1))
                        for u2 in range(2):
                            nc.scalar.activation(
                                h2[s][:, u2 * CH:(u2 + 1) * CH], ps2[u2][:],
                                ACTF.Relu, bias=b2t[:, u2:u2 + 1])

                        # G3 + fused pair-sum into acc column
                        for u2 in range(2):
                            for k in range(2):
                                nc.tensor.matmul(ps3[u2][:], w3[k][u2][:],
                                                 h2[s][:, k * CH:(k + 1) * CH],
                                                 start=(k == 0), stop=(k == 1))
                        # u2=0 on DVE: sum max(x,-b3) (fb1 corrected on host)
                        nc.vector.tensor_scalar(ps3[0][:], ps3[0][:],
                                                b3t[:, 0:1], None,
                                                op0=ALU.max, op1=ALU.add,
                                                accum_out=acc[0][:, c:c + 1])
                        # u2=1 on ACT: fused relu + accumulate
                        nc.scalar.activation(ps3[1][:], ps3[1][:], ACTF.Relu,
                                             bias=b3t[:, 1:2],
                                             accum_out=acc[1][:, c:c + 1])

                # ---- pooled sum per batch element: [128, BC] per u-chunk ----
                for u2 in range(2):
                    accv = acc[u2][:].rearrange("p (b c) -> p b c", c=NCHUNK_B)
                    nc.vector.reduce_sum(pooled[u2][:], accv, axis=AX.X)

                # ---- F head (full fp32 matmuls; tiny) ----
                for u2 in range(2):
                    psf = pb[u2]
                    for k in range(2):
                        nc.tensor.matmul(psf[:, :BC], fw1[k][u2][:],
                                         pooled[k][:],
                                         start=(k == 0), stop=(k == 1))
                    nc.scalar.activation(hf1[u2][:], psf[:, :BC], ACTF.Relu,
                                         bias=fb1t[:, u2:u2 + 1])
                for u2 in range(2):
                    psf = pb[2 + u2]
                    for k in range(2):
                        nc.tensor.matmul(psf[:, :BC], fw2[k][u2][:],
                                         hf1[k][:],
                                         start=(k == 0), stop=(k == 1))
                    nc.scalar.activation(hf2[u2][:], psf[:, :BC], ACTF.Relu,
                                         bias=fb2t[:, u2:u2 + 1])
                # out[b, o] = sum_t hf2^T[t, b] * foW^T[t, o] + fo_b
                pso = pb[4]
                for k in range(2):
                    nc.tensor.matmul(pso[:BC, :OUT], hf2[k][:], fow[k][:],
                                     start=(k == 0), stop=(k == 1))
                nc.vector.tensor_tensor(outt[:], pso[:BC, :OUT], fobt[:],
                                        op=ALU.add)
                nc.sync.dma_start(out_d[:], outt[:])

    nc.compile()
    return nc


def _prep(inputs):
    """Host-side: fold BN into weights, transpose to device layouts."""
    g_W, g_b = inputs["g_W"], inputs["g_b"]
    f_W, f_b = inputs["f_W"], inputs["f_b"]

    def bn_fold(W, bvec, gamma, beta, mean, var):
        s = gamma / np.sqrt(var + EPS)
        We = s[:, None] * W
        be = s * bvec + beta - mean * s
        return We.astype(np.float32), be.astype(np.float32)

    def cols2(v):  # [T] -> [128, 2]
        return np.stack([v[:128], v[128:]], axis=1).astype(np.float32)

    def tile4(Wt):  # [T, T] (t, u) -> [2, 2, 128, 128] = [k, u2, t_in, u_in]
        return np.ascontiguousarray(
            Wt.reshape(2, 128, 2, 128).transpose(0, 2, 1, 3)).astype(np.float32)

    W0e, b1e = bn_fold(g_W[0], g_b[0], inputs["g_gamma"][0], inputs["g_beta"][0],
                       inputs["g_mean"][0], inputs["g_var"][0])
    W2e, b2e = bn_fold(g_W[1], g_b[1], inputs["g_gamma"][1], inputs["g_beta"][1],
                       inputs["g_mean"][1], inputs["g_var"][1])
    W3e, b3e = bn_fold(g_W[2], g_b[2], inputs["g_gamma"][2], inputs["g_beta"][2],
                       inputs["g_mean"][2], inputs["g_var"][2])
    F1e, fb1e = bn_fold(f_W[0], f_b[0], inputs["f_gamma"][0], inputs["f_beta"][0],
                        inputs["f_mean"][0], inputs["f_var"][0])
    F2e, fb2e = bn_fold(f_W[1], f_b[1], inputs["f_gamma"][1], inputs["f_beta"][1],
                        inputs["f_mean"][1], inputs["f_var"][1])
    # the DVE max-trick pair-sum for the u2=0 half omits +b3 per element;
    # compensate in the F1 bias: pooled_true = pooled_raw + N*N*b3 (half 0)
    corr = np.zeros(T, dtype=np.float64)
    corr[:128] = float(N * N) * b3e[:128].astype(np.float64)
    fb1e = (fb1e.astype(np.float64) + F1e.astype(np.float64) @ corr).astype(np.float32)

    shared = {
        "w0l": np.ascontiguousarray(W0e[:, :D].T),
        "w0r": np.ascontiguousarray(W0e[:, D:].T),
        "b1": cols2(b1e),
        "w2": tile4(W2e.T), "b2": cols2(b2e),
        "w3": tile4(W3e.T),
        "b3": np.stack([-b3e[:128], b3e[128:]], axis=1).astype(np.float32),
        "fw1": tile4(F1e.T), "fb1": cols2(fb1e),
        "fw2": tile4(F2e.T), "fb2": cols2(fb2e),
        "fow": np.ascontiguousarray(
            inputs["fo_W"].T.reshape(2, 128, OUT)).astype(np.float32),
        "fob": np.tile(inputs["fo_b"].reshape(1, OUT), (BC, 1)).astype(np.float32),
    }
    shared["w0l"] = shared["w0l"].astype(np.float16)
    shared["w0r"] = shared["w0r"].astype(np.float16)
    x = inputs["x"]
    in_maps = []
    for c in range(N_CORES):
        xc = x[c * BC:(c + 1) * BC]  # [BC, N, D]
        xt = np.ascontiguousarray(xc.transpose(2, 0, 1).reshape(D, BC * N))
        in_maps.append({"xt": xt.astype(np.float16), **shared})
    return in_maps


def kernel(**inputs):
    global LAST
    inputs = {k: np.asarray(v, dtype=np.float32) for k, v in inputs.items()}
    if "nc" not in _CACHE:
        _CACHE["nc"] = _build()
    nc = _CACHE["nc"]
    in_maps = _prep(inputs)
    res = run_bass_kernel_spmd(nc, in_maps, core_ids=list(range(N_CORES)),
                               trace=TRACE)
    LAST = res
    out = np.concatenate([res.results[c]["out"] for c in range(N_CORES)], axis=0)
    return out.astype(np.float32)


# revision 47
# speedup vs baseline: 1.0804x; 1.0582x over previous
"""RelationNetwork (B=32, N=64, D=128) Bass/Tile kernel for 8 TRN2 NeuronCores.

Strategy: pure data-parallel over the batch (4 batch elements/core).
  - BN (eval mode) is folded into the Linear weights/biases on the host.
  - Layer G1 is separable: h1[i,j] = relu(A'_i + R_j) with A = x @ W0L'^T + b1,
    R = x @ W0R'^T computed once per core (N rows instead of N^2).
  - Layers G2/G3 are dense matmuls over the 4096-pair axis, processed in
    512-pair chunks held entirely on-chip (feature-major layout [T, pairs]).
  - The pair-sum is fused into the G3 PSUM evacuations (ACT accum_out on one
    half; a DVE max-trick accumulate on the other, bias-corrected in fb1).
  - The tiny F head runs on-device on the pooled [T, 4] activations.
Matmuls for G2/G3 use float32r (full PE rate at free-dim 512, fp32 storage).
All tiles are statically allocated (manual slot rotation, no pool recycling):
on this execution environment, tile-pool alloc/release machinery costs
~100+us per event, dominating everything else.
"""
import numpy as np

import concourse.bass as bass
import concourse.tile as tile
from concourse import bacc, mybir
from concourse.bass_utils import run_bass_kernel_spmd

N_CORES = 8
B, N, D = 32, 64, 128
T = 2 * D            # 256
OUT = 128
BC = B // N_CORES    # batch elements per core = 4
IC = 8               # i-values per pair chunk  -> chunk = IC*N = 512 pairs
NCHUNK_B = N // IC   # chunks per batch element = 8
NCHUNK = BC * NCHUNK_B  # 32 chunks per core
CH = IC * N          # 512
EPS = 1e-5
NSLOT = 8            # static slot-ring depth for h1/h2 buffers

F32 = mybir.dt.float32
F32R = mybir.dt.float32r
F16 = mybir.dt.float16
AX = mybir.AxisListType
ALU = mybir.AluOpType
ACTF = mybir.ActivationFunctionType

_CACHE: dict = {}
TRACE = False
LAST = None  # BassKernelResults of the most recent run (for profiling)


def _build(repeat=1):
    nc = bacc.Bacc("TRN2", target_bir_lowering=False, debug=False,
                   enable_asserts=False, num_devices=N_CORES)

    def din(name, shape, dt=F32):
        return nc.dram_tensor(name, list(shape), dt, kind="ExternalInput").ap()

    xt_d = din("xt", (128, BC * N), F16)           # x^T per core: [d, (b,i)]
    w0l_d = din("w0l", (128, T), F16)              # (s0*W0[:, :D])^T  [d, u]
    w0r_d = din("w0r", (128, T), F16)
    w2_d = din("w2", (2, 2, 128, 128), F32R)       # [k, u2, t_in, u_in]
    w3_d = din("w3", (2, 2, 128, 128), F32R)
    b1_d = din("b1", (128, 2))               # col k = bias chunk k
    b2_d = din("b2", (128, 2))
    b3_d = din("b3", (128, 3))   # cols: -b3[:128], +b3[128:], -b3[128:]
    fw1_d = din("fw1", (2, 2, 128, 128))
    fw2_d = din("fw2", (2, 2, 128, 128))
    fb1_d = din("fb1", (128, 2))
    fb2_d = din("fb2", (128, 2))
    fow_d = din("fow", (2, 128, 128))        # fo_W^T tiled [k, t_in, o]
    fob_d = din("fob", (BC, OUT))            # fo_b replicated across BC rows
    out_d = nc.dram_tensor("out", [BC, OUT], F32, kind="ExternalOutput").ap()

    with tile.TileContext(nc) as tc:
        with (
            tc.tile_pool(name="wpool", bufs=1) as wp,
            tc.tile_pool(name="pspool", bufs=1, space="PSUM") as psp,
        ):
            # ---- persistent loads ----
            def load(dram_ap, shape, tag, dt=F32):
                t = wp.tile(list(shape), dt, tag=tag, name=tag)
                nc.sync.dma_start(t[:], dram_ap)
                return t

            # warm the ACT function table during the DMA window (the
            # implicit LoadActFuncSet otherwise lands on the startup
            # critical chain, ~1.3us)
            dummy = wp.tile([1, 2], F32, tag="dummy", name="dummy")
            nc.gpsimd.memset(dummy[:], 0.0)
            nc.scalar.activation(dummy[:, 0:1], dummy[:, 0:1], ACTF.Relu,
                                 bias=0.0)

            xt = load(xt_d[:], (128, BC * N), "xt", F16)
            w0l = load(w0l_d[:], (128, T), "w0l", F16)
            w0r = load(w0r_d[:], (128, T), "w0r", F16)
            b1t = load(b1_d[:], (128, 2), "b1t")
            b2t = load(b2_d[:], (128, 2), "b2t")
            b3t = load(b3_d[:], (128, 3), "b3t")
            w2 = [[load(w2_d[k, u2], (128, 128), f"w2_{k}{u2}", F32R)
                   for u2 in range(2)] for k in range(2)]
            w3 = [[load(w3_d[k, u2], (128, 128), f"w3_{k}{u2}", F32R)
                   for u2 in range(2)] for k in range(2)]
            fw1 = [[load(fw1_d[k, u2], (128, 128), f"fw1_{k}{u2}")
                    for u2 in range(2)] for k in range(2)]
            fw2 = [[load(fw2_d[k, u2], (128, 128), f"fw2_{k}{u2}")
                    for u2 in range(2)] for k in range(2)]
            fow = [load(fow_d[k], (128, 128), f"fow_{k}") for k in range(2)]
            fb1t = load(fb1_d[:], (128, 2), "fb1t")
            fb2t = load(fb2_d[:], (128, 2), "fb2t")
            fobt = load(fob_d[:], (BC, OUT), "fobt")

            # static compute buffers (no recycling)
            at = wp.tile([128, 2 * BC * N], F32, tag="at", name="at")
            rt = wp.tile([128, 2 * BC * N], F32, tag="rt", name="rt")
            h1pre = [wp.tile([128, 2 * CH], F32, tag=f"h1p{j}", name=f"h1p{j}")
                     for j in range(NSLOT)]
            h1 = [wp.tile([128, 2 * CH], F32R, tag=f"h1_{j}", name=f"h1_{j}")
                  for j in range(NSLOT)]
            h2 = [wp.tile([128, 2 * CH], F32R, tag=f"h2_{j}", name=f"h2_{j}")
                  for j in range(NSLOT)]
            acc = [wp.tile([128, NCHUNK], F32, tag=f"acc{u2}", name=f"acc{u2}")
                   for u2 in range(2)]
            pooled = [wp.tile([128, BC], F32, tag=f"pl{u2}", name=f"pl{u2}")
                      for u2 in range(2)]
            hf1 = [wp.tile([128, BC], F32, tag=f"hf1{u2}", name=f"hf1{u2}")
                   for u2 in range(2)]
            hf2 = [wp.tile([128, BC], F32, tag=f"hf2{u2}", name=f"hf2{u2}")
                   for u2 in range(2)]
            outt = wp.tile([BC, OUT], F32, tag="outt", name="outt")
            # 8 static PSUM banks, rotated manually
            pb = [psp.tile([128, CH], F32, tag=f"pb{j}", name=f"pb{j}")
                  for j in range(8)]

            for _ in range(repeat):
                # ---- G1 setup: A' = x @ W0L'^T + b1 ; R = x @ W0R'^T ----
                # at/rt cols = u2*256 + (b*64 + i)
                for u2 in range(2):
                    psa = pb[u2]
                    nc.tensor.matmul(psa[:, :BC * N],
                                     w0l[:, u2 * 128:(u2 + 1) * 128],
                                     xt[:], start=True, stop=True)
                    nc.scalar.activation(at[:, u2 * BC * N:(u2 + 1) * BC * N],
                                         psa[:, :BC * N], ACTF.Identity,
                                         bias=b1t[:, u2:u2 + 1])
                    psr = pb[2 + u2]
                    nc.tensor.matmul(psr[:, :BC * N],
                                     w0r[:, u2 * 128:(u2 + 1) * 128],
                                     xt[:], start=True, stop=True)
                    nc.vector.tensor_copy(rt[:, u2 * BC * N:(u2 + 1) * BC * N],
                                          psr[:, :BC * N])

                # ---- main loop over 512-pair chunks ----
                for b in range(BC):
                    for ic in range(NCHUNK_B):
                        c = b * NCHUNK_B + ic
                        s = c % NSLOT
                        ps2 = [pb[(4 * c) % 8], pb[(4 * c + 1) % 8]]
                        ps3 = [pb[(4 * c + 2) % 8], pb[(4 * c + 3) % 8]]
                        # h1[:, k*512 + i*64 + j] = relu(A'[u,i] + R'[u,j])
                        a_sl = (at[:].rearrange("p (v c) -> p v c", v=2)
                                [:, :, b * N + ic * IC: b * N + (ic + 1) * IC])
                        a_b = a_sl.unsqueeze(3).broadcast_to((128, 2, IC, N))
                        r_sl = (rt[:].rearrange("p (v c) -> p v c", v=2)
                                [:, :, b * N:(b + 1) * N])
                        r_b = r_sl.unsqueeze(2).broadcast_to((128, 2, IC, N))
                        h1v = h1pre[s][:].rearrange("p (v a b) -> p v a b",
                                                    v=2, b=N)
                        # engine-load balancing: a few chunks' adds run on
                        # GPSIMD (slower there, but ACT/DVE are the bottleneck)
                        tteng = nc.gpsimd if c % 7 == 3 else nc.vector
                        tteng.tensor_tensor(h1v, a_b, r_b, op=ALU.add)
                        nc.gpsimd.tensor_scalar_max(h1[s][:], h1pre[s][:], 0.0)

                        # G2: ps2[u2] = sum_k w2[k][u2]^T @ h1[k]
                        for u2 in range(2):
                            for k in range(2):
                                nc.tensor.matmul(ps2[u2][:], w2[k][u2][:],
                                                 h1[s][:, k * CH:(k + 1) * CH],
                                                 start=(k == 0), stop=(k == 1))
                        for u2 in range(2):
                            nc.scalar.activation(
                                h2[s][:, u2 * CH:(u2 + 1) * CH], ps2[u2][:],
                                ACTF.Relu, bias=b2t[:, u2:u2 + 1])

                        # G3 + fused pair-sum into acc column
                        for u2 in range(2):
                            for k in range(2):
                                nc.tensor.matmul(ps3[u2][:], w3[k][u2][:],
                                                 h2[s][:, k * CH:(k + 1) * CH],
                                                 start=(k == 0), stop=(k == 1))
                        # u2=0 on DVE: sum max(x,-b3) (fb1 corrected on host)
                        nc.vector.tensor_scalar(ps3[0][:], ps3[0][:],
                                                b3t[:, 0:1], None,
                                                op0=ALU.max, op1=ALU.add,
                                                accum_out=acc[0][:, c:c + 1])
                        # u2=1: mostly ACT (fused relu+accumulate); two chunks
                        # per batch element go to DVE via the max-trick (the
                        # omitted +b3 constant is folded into fb1 on the host)
                        if ic in (3, 6):
                            nc.vector.tensor_scalar(ps3[1][:], ps3[1][:],
                                                    b3t[:, 2:3], None,
                                                    op0=ALU.max, op1=ALU.add,
                                                    accum_out=acc[1][:, c:c + 1])
                        else:
                            nc.scalar.activation(ps3[1][:], ps3[1][:], ACTF.Relu,
                                                 bias=b3t[:, 1:2],
                                                 accum_out=acc[1][:, c:c + 1])

                # ---- pooled sum per batch element: [128, BC] per u-chunk ----
                for u2 in range(2):
                    accv = acc[u2][:].rearrange("p (b c) -> p b c", c=NCHUNK_B)
                    nc.vector.reduce_sum(pooled[u2][:], accv, axis=AX.X)

                # ---- F head (full fp32 matmuls; tiny) ----
                for u2 in range(2):
                    psf = pb[u2]
                    for k in range(2):
                        nc.tensor.matmul(psf[:, :BC], fw1[k][u2][:],
                                         pooled[k][:],
                                         start=(k == 0), stop=(k == 1))
                    nc.scalar.activation(hf1[u2][:], psf[:, :BC], ACTF.Relu,
                                         bias=fb1t[:, u2:u2 + 1])
                for u2 in range(2):
                    psf = pb[2 + u2]
                    for k in range(2):
                        nc.tensor.matmul(psf[:, :BC], fw2[k][u2][:],
                                         hf1[k][:],
                                         start=(k == 0), stop=(k == 1))
                    nc.scalar.activation(hf2[u2][:], psf[:, :BC], ACTF.Relu,
                                         bias=fb2t[:, u2:u2 + 1])
                # out[b, o] = sum_t hf2^T[t, b] * foW^T[t, o] + fo_b
                pso = pb[4]
                for k in range(2):
                    nc.tensor.matmul(pso[:BC, :OUT], hf2[k][:], fow[k][:],
                                     start=(k == 0), stop=(k == 1))
                nc.vector.tensor_tensor(outt[:], pso[:BC, :OUT], fobt[:],
                                        op=ALU.add)
                nc.sync.dma_start(out_d[:], outt[:])

    nc.compile()
    return nc


def _prep(inputs):
    """Host-side: fold BN into weights, transpose to device layouts."""
    g_W, g_b = inputs["g_W"], inputs["g_b"]
    f_W, f_b = inputs["f_W"], inputs["f_b"]

    def bn_fold(W, bvec, gamma, beta, mean, var):
        s = gamma / np.sqrt(var + EPS)
        We = s[:, None] * W
        be = s * bvec + beta - mean * s
        return We.astype(np.float32), be.astype(np.float32)

    def cols2(v):  # [T] -> [128, 2]
        return np.stack([v[:128], v[128:]], axis=1).astype(np.float32)

    def tile4(Wt):  # [T, T] (t, u) -> [2, 2, 128, 128] = [k, u2, t_in, u_in]
        return np.ascontiguousarray(
            Wt.reshape(2, 128, 2, 128).transpose(0, 2, 1, 3)).astype(np.float32)

    W0e, b1e = bn_fold(g_W[0], g_b[0], inputs["g_gamma"][0], inputs["g_beta"][0],
                       inputs["g_mean"][0], inputs["g_var"][0])
    W2e, b2e = bn_fold(g_W[1], g_b[1], inputs["g_gamma"][1], inputs["g_beta"][1],
                       inputs["g_mean"][1], inputs["g_var"][1])
    W3e, b3e = bn_fold(g_W[2], g_b[2], inputs["g_gamma"][2], inputs["g_beta"][2],
                       inputs["g_mean"][2], inputs["g_var"][2])
    F1e, fb1e = bn_fold(f_W[0], f_b[0], inputs["f_gamma"][0], inputs["f_beta"][0],
                        inputs["f_mean"][0], inputs["f_var"][0])
    F2e, fb2e = bn_fold(f_W[1], f_b[1], inputs["f_gamma"][1], inputs["f_beta"][1],
                        inputs["f_mean"][1], inputs["f_var"][1])
    # the DVE max-trick pair-sums omit +b3 per element; compensate in the F1
    # bias. Half u2=0: all 4096 pairs per batch go through DVE. Half u2=1:
    # 2 of the 8 chunks per batch (= 1024 pairs) go through DVE.
    corr = np.zeros(T, dtype=np.float64)
    corr[:128] = float(N * N) * b3e[:128].astype(np.float64)
    corr[128:] = float(2 * CH) * b3e[128:].astype(np.float64)
    fb1e = (fb1e.astype(np.float64) + F1e.astype(np.float64) @ corr).astype(np.float32)

    shared = {
        "w0l": np.ascontiguousarray(W0e[:, :D].T),
        "w0r": np.ascontiguousarray(W0e[:, D:].T),
        "b1": cols2(b1e),
        "w2": tile4(W2e.T), "b2": cols2(b2e),
        "w3": tile4(W3e.T),
        "b3": np.stack([-b3e[:128], b3e[128:], -b3e[128:]],
                       axis=1).astype(np.float32),
        "fw1": tile4(F1e.T), "fb1": cols2(fb1e),
        "fw2": tile4(F2e.T), "fb2": cols2(fb2e),
        "fow": np.ascontiguousarray(
            inputs["fo_W"].T.reshape(2, 128, OUT)).astype(np.float32),
        "fob": np.tile(inputs["fo_b"].reshape(1, OUT), (BC, 1)).astype(np.float32),
    }
    shared["w0l"] = shared["w0l"].astype(np.float16)
    shared["w0r"] = shared["w0r"].astype(np.float16)
    x = inputs["x"]
    in_maps = []
    for c in range(N_CORES):
        xc = x[c * BC:(c + 1) * BC]  # [BC, N, D]
        xt = np.ascontiguousarray(xc.transpose(2, 0, 1).reshape(D, BC * N))
        in_maps.append({"xt": xt.astype(np.float16), **shared})
    return in_maps


def kernel(**inputs):
    global LAST
    inputs = {k: np.asarray(v, dtype=np.float32) for k, v in inputs.items()}
    if "nc" not in _CACHE:
        _CACHE["nc"] = _build()
    nc = _CACHE["nc"]
    in_maps = _prep(inputs)
    res = run_bass_kernel_spmd(nc, in_maps, core_ids=list(range(N_CORES)),
                               trace=TRACE)
    LAST = res
    out = np.concatenate([res.results[c]["out"] for c in range(N_CORES)], axis=0)
    return out.astype(np.float32)


# revision 50
# speedup vs baseline: 1.0910x; 1.0098x over previous
"""RelationNetwork (B=32, N=64, D=128) Bass/Tile kernel for 8 TRN2 NeuronCores.

Strategy: pure data-parallel over the batch (4 batch elements/core).
  - BN (eval mode) is folded into the Linear weights/biases on the host.
  - Layer G1 is separable: h1[i,j] = relu(A'_i + R_j) with A = x @ W0L'^T + b1,
    R = x @ W0R'^T computed once per core (N rows instead of N^2).
  - Layers G2/G3 are dense matmuls over the 4096-pair axis, processed in
    512-pair chunks held entirely on-chip (feature-major layout [T, pairs]).
  - The pair-sum is fused into the G3 PSUM evacuations (ACT accum_out on one
    half; a DVE max-trick accumulate on the other, bias-corrected in fb1).
  - The tiny F head runs on-device on the pooled [T, 4] activations.
Matmuls for G2/G3 use float32r (full PE rate at free-dim 512, fp32 storage).
All tiles are statically allocated (manual slot rotation, no pool recycling):
on this execution environment, tile-pool alloc/release machinery costs
~100+us per event, dominating everything else.
"""
import numpy as np

import concourse.bass as bass
import concourse.tile as tile
from concourse import bacc, mybir
from concourse.bass_utils import run_bass_kernel_spmd

N_CORES = 8
B, N, D = 32, 64, 128
T = 2 * D            # 256
OUT = 128
BC = B // N_CORES    # batch elements per core = 4
IC = 8               # i-values per pair chunk  -> chunk = IC*N = 512 pairs
NCHUNK_B = N // IC   # chunks per batch element = 8
NCHUNK = BC * NCHUNK_B  # 32 chunks per core
CH = IC * N          # 512
EPS = 1e-5
NSLOT = 8            # static slot-ring depth for h1/h2 buffers

F32 = mybir.dt.float32
F32R = mybir.dt.float32r
F16 = mybir.dt.float16
AX = mybir.AxisListType
ALU = mybir.AluOpType
ACTF = mybir.ActivationFunctionType

_CACHE: dict = {}
TRACE = False
LAST = None  # BassKernelResults of the most recent run (for profiling)


def _build(repeat=1):
    nc = bacc.Bacc("TRN2", target_bir_lowering=False, debug=False,
                   enable_asserts=False, num_devices=N_CORES)

    def din(name, shape, dt=F32):
        return nc.dram_tensor(name, list(shape), dt, kind="ExternalInput").ap()

    xt_d = din("xt", (128, BC * N), F16)           # x^T per core: [d, (b,i)]
    w0l_d = din("w0l", (128, T), F16)              # (s0*W0[:, :D])^T  [d, u]
    w0r_d = din("w0r", (128, T), F16)
    w2_d = din("w2", (2, 2, 128, 128), F32R)       # [k, u2, t_in, u_in]
    w3_d = din("w3", (2, 2, 128, 128), F32R)
    b1_d = din("b1", (128, 2))               # col k = bias chunk k
    b2_d = din("b2", (128, 2))
    b3_d = din("b3", (128, 3))   # cols: -b3[:128], +b3[128:], -b3[128:]
    fw1_d = din("fw1", (2, 2, 128, 128))
    fw2_d = din("fw2", (2, 2, 128, 128))
    fb1_d = din("fb1", (128, 2))
    fb2_d = din("fb2", (128, 2))
    fow_d = din("fow", (2, 128, 128))        # fo_W^T tiled [k, t_in, o]
    fob_d = din("fob", (BC, OUT))            # fo_b replicated across BC rows
    out_d = nc.dram_tensor("out", [BC, OUT], F32, kind="ExternalOutput").ap()

    with tile.TileContext(nc) as tc:
        with (
            tc.tile_pool(name="wpool", bufs=1) as wp,
            tc.tile_pool(name="pspool", bufs=1, space="PSUM") as psp,
        ):
            # ---- persistent loads ----
            def load(dram_ap, shape, tag, dt=F32):
                t = wp.tile(list(shape), dt, tag=tag, name=tag)
                nc.sync.dma_start(t[:], dram_ap)
                return t

            # warm the ACT function table during the DMA window (the
            # implicit LoadActFuncSet otherwise lands on the startup
            # critical chain, ~1.3us)
            dummy = wp.tile([1, 2], F32, tag="dummy", name="dummy")
            nc.gpsimd.memset(dummy[:], 0.0)
            nc.scalar.activation(dummy[:, 0:1], dummy[:, 0:1], ACTF.Relu,
                                 bias=0.0)

            xt = load(xt_d[:], (128, BC * N), "xt", F16)
            w0l = load(w0l_d[:], (128, T), "w0l", F16)
            w0r = load(w0r_d[:], (128, T), "w0r", F16)
            b1t = load(b1_d[:], (128, 2), "b1t")
            b2t = load(b2_d[:], (128, 2), "b2t")
            b3t = load(b3_d[:], (128, 3), "b3t")
            w2 = [[load(w2_d[k, u2], (128, 128), f"w2_{k}{u2}", F32R)
                   for u2 in range(2)] for k in range(2)]
            w3 = [[load(w3_d[k, u2], (128, 128), f"w3_{k}{u2}", F32R)
                   for u2 in range(2)] for k in range(2)]
            fw1 = [[load(fw1_d[k, u2], (128, 128), f"fw1_{k}{u2}")
                    for u2 in range(2)] for k in range(2)]
            fw2 = [[load(fw2_d[k, u2], (128, 128), f"fw2_{k}{u2}")
                    for u2 in range(2)] for k in range(2)]
            fow = [load(fow_d[k], (128, 128), f"fow_{k}") for k in range(2)]
            fb1t = load(fb1_d[:], (128, 2), "fb1t")
            fb2t = load(fb2_d[:], (128, 2), "fb2t")
            fobt = load(fob_d[:], (BC, OUT), "fobt")

            # static compute buffers (no recycling)
            at = wp.tile([128, 2 * BC * N], F32, tag="at", name="at")
            rt = wp.tile([128, 2 * BC * N], F32, tag="rt", name="rt")
            h1pre = [wp.tile([128, 2 * CH], F32, tag=f"h1p{j}", name=f"h1p{j}")
                     for j in range(NSLOT)]
            h1 = [wp.tile([128, 2 * CH], F32R, tag=f"h1_{j}", name=f"h1_{j}")
                  for j in range(NSLOT)]
            h2 = [wp.tile([128, 2 * CH], F32R, tag=f"h2_{j}", name=f"h2_{j}")
                  for j in range(NSLOT)]
            acc = [wp.tile([128, NCHUNK], F32, tag=f"acc{u2}", name=f"acc{u2}")
                   for u2 in range(2)]
            pooled = [wp.tile([128, BC], F32, tag=f"pl{u2}", name=f"pl{u2}")
                      for u2 in range(2)]
            hf1 = [wp.tile([128, BC], F32, tag=f"hf1{u2}", name=f"hf1{u2}")
                   for u2 in range(2)]
            hf2 = [wp.tile([128, BC], F32, tag=f"hf2{u2}", name=f"hf2{u2}")
                   for u2 in range(2)]
            outt = wp.tile([BC, OUT], F32, tag="outt", name="outt")
            # 8 static PSUM banks, rotated manually
            pb = [psp.tile([128, CH], F32, tag=f"pb{j}", name=f"pb{j}")
                  for j in range(8)]

            for _ in range(repeat):
                # ---- G1 setup: A' = x @ W0L'^T + b1 ; R = x @ W0R'^T ----
                # at/rt cols = u2*256 + (b*64 + i)
                for u2 in range(2):
                    psa = pb[u2]
                    nc.tensor.matmul(psa[:, :BC * N],
                                     w0l[:, u2 * 128:(u2 + 1) * 128],
                                     xt[:], start=True, stop=True)
                    nc.scalar.activation(at[:, u2 * BC * N:(u2 + 1) * BC * N],
                                         psa[:, :BC * N], ACTF.Identity,
                                         bias=b1t[:, u2:u2 + 1])
                    psr = pb[2 + u2]
                    nc.tensor.matmul(psr[:, :BC * N],
                                     w0r[:, u2 * 128:(u2 + 1) * 128],
                                     xt[:], start=True, stop=True)
                    nc.vector.tensor_copy(rt[:, u2 * BC * N:(u2 + 1) * BC * N],
                                          psr[:, :BC * N])

                # ---- main loop over 512-pair chunks ----
                for b in range(BC):
                    for ic in range(NCHUNK_B):
                        c = b * NCHUNK_B + ic
                        s = c % NSLOT
                        ps2 = [pb[(4 * c) % 8], pb[(4 * c + 1) % 8]]
                        ps3 = [pb[(4 * c + 2) % 8], pb[(4 * c + 3) % 8]]
                        # h1[:, k*512 + i*64 + j] = relu(A'[u,i] + R'[u,j])
                        a_sl = (at[:].rearrange("p (v c) -> p v c", v=2)
                                [:, :, b * N + ic * IC: b * N + (ic + 1) * IC])
                        a_b = a_sl.unsqueeze(3).broadcast_to((128, 2, IC, N))
                        r_sl = (rt[:].rearrange("p (v c) -> p v c", v=2)
                                [:, :, b * N:(b + 1) * N])
                        r_b = r_sl.unsqueeze(2).broadcast_to((128, 2, IC, N))
                        h1v = h1pre[s][:].rearrange("p (v a b) -> p v a b",
                                                    v=2, b=N)
                        # engine-load balancing: a few chunks' adds run on
                        # GPSIMD (slower there, but ACT/DVE are the bottleneck)
                        tteng = nc.gpsimd if c % 7 == 3 else nc.vector
                        tteng.tensor_tensor(h1v, a_b, r_b, op=ALU.add)
                        # first chunks' relu on DVE (idle during pipeline
                        # fill; shortens the startup critical chain)
                        reng = nc.vector if c < 2 else nc.gpsimd
                        reng.tensor_scalar_max(h1[s][:], h1pre[s][:], 0.0)

                        # G2: ps2[u2] = sum_k w2[k][u2]^T @ h1[k]
                        for u2 in range(2):
                            for k in range(2):
                                nc.tensor.matmul(ps2[u2][:], w2[k][u2][:],
                                                 h1[s][:, k * CH:(k + 1) * CH],
                                                 start=(k == 0), stop=(k == 1))
                        for u2 in range(2):
                            nc.scalar.activation(
                                h2[s][:, u2 * CH:(u2 + 1) * CH], ps2[u2][:],
                                ACTF.Relu, bias=b2t[:, u2:u2 + 1])

                        # G3 + fused pair-sum into acc column
                        for u2 in range(2):
                            for k in range(2):
                                nc.tensor.matmul(ps3[u2][:], w3[k][u2][:],
                                                 h2[s][:, k * CH:(k + 1) * CH],
                                                 start=(k == 0), stop=(k == 1))
                        # u2=0 on DVE: sum max(x,-b3) (fb1 corrected on host)
                        nc.vector.tensor_scalar(ps3[0][:], ps3[0][:],
                                                b3t[:, 0:1], None,
                                                op0=ALU.max, op1=ALU.add,
                                                accum_out=acc[0][:, c:c + 1])
                        # u2=1: mostly ACT (fused relu+accumulate); two chunks
                        # per batch element go to DVE via the max-trick (the
                        # omitted +b3 constant is folded into fb1 on the host)
                        if ic in (3, 6):
                            nc.vector.tensor_scalar(ps3[1][:], ps3[1][:],
                                                    b3t[:, 2:3], None,
                                                    op0=ALU.max, op1=ALU.add,
                                                    accum_out=acc[1][:, c:c + 1])
                        else:
                            nc.scalar.activation(ps3[1][:], ps3[1][:], ACTF.Relu,
                                                 bias=b3t[:, 1:2],
                                                 accum_out=acc[1][:, c:c + 1])

                # ---- pooled sum per batch element: [128, BC] per u-chunk ----
                for u2 in range(2):
                    accv = acc[u2][:].rearrange("p (b c) -> p b c", c=NCHUNK_B)
                    nc.vector.reduce_sum(pooled[u2][:], accv, axis=AX.X)

                # ---- F head (full fp32 matmuls; tiny) ----
                for u2 in range(2):
                    psf = pb[u2]
                    for k in range(2):
                        nc.tensor.matmul(psf[:, :BC], fw1[k][u2][:],
                                         pooled[k][:],
                                         start=(k == 0), stop=(k == 1))
                    nc.scalar.activation(hf1[u2][:], psf[:, :BC], ACTF.Relu,
                                         bias=fb1t[:, u2:u2 + 1])
                for u2 in range(2):
                    psf = pb[2 + u2]
                    for k in range(2):
                        nc.tensor.matmul(psf[:, :BC], fw2[k][u2][:],
                                         hf1[k][:],
                                         start=(k == 0), stop=(k == 1))
                    nc.scalar.activation(hf2[u2][:], psf[:, :BC], ACTF.Relu,
                                         bias=fb2t[:, u2:u2 + 1])
                # out[b, o] = sum_t hf2^T[t, b] * foW^T[t, o] + fo_b
                pso = pb[4]
                for k in range(2):
                    nc.tensor.matmul(pso[:BC, :OUT], hf2[k][:], fow[k][:],
                                     start=(k == 0), stop=(k == 1))
                nc.vector.tensor_tensor(outt[:], pso[:BC, :OUT], fobt[:],
                                        op=ALU.add)
                nc.sync.dma_start(out_d[:], outt[:])

    nc.compile()
    return nc


def _prep(inputs):
    """Host-side: fold BN into weights, transpose to device layouts."""
    g_W, g_b = inputs["g_W"], inputs["g_b"]
    f_W, f_b = inputs["f_W"], inputs["f_b"]

    def bn_fold(W, bvec, gamma, beta, mean, var):
        s = gamma / np.sqrt(var + EPS)
        We = s[:, None] * W
        be = s * bvec + beta - mean * s
        return We.astype(np.float32), be.astype(np.float32)

    def cols2(v):  # [T] -> [128, 2]
        return np.stack([v[:128], v[128:]], axis=1).astype(np.float32)

    def tile4(Wt):  # [T, T] (t, u) -> [2, 2, 128, 128] = [k, u2, t_in, u_in]
        return np.ascontiguousarray(
            Wt.reshape(2, 128, 2, 128).transpose(0, 2, 1, 3)).astype(np.float32)

    W0e, b1e = bn_fold(g_W[0], g_b[0], inputs["g_gamma"][0], inputs["g_beta"][0],
                       inputs["g_mean"][0], inputs["g_var"][0])
    W2e, b2e = bn_fold(g_W[1], g_b[1], inputs["g_gamma"][1], inputs["g_beta"][1],
                       inputs["g_mean"][1], inputs["g_var"][1])
    W3e, b3e = bn_fold(g_W[2], g_b[2], inputs["g_gamma"][2], inputs["g_beta"][2],
                       inputs["g_mean"][2], inputs["g_var"][2])
    F1e, fb1e = bn_fold(f_W[0], f_b[0], inputs["f_gamma"][0], inputs["f_beta"][0],
                        inputs["f_mean"][0], inputs["f_var"][0])
    F2e, fb2e = bn_fold(f_W[1], f_b[1], inputs["f_gamma"][1], inputs["f_beta"][1],
                        inputs["f_mean"][1], inputs["f_var"][1])
    # the DVE max-trick pair-sums omit +b3 per element; compensate in the F1
    # bias. Half u2=0: all 4096 pairs per batch go through DVE. Half u2=1:
    # 2 of the 8 chunks per batch (= 1024 pairs) go through DVE.
    corr = np.zeros(T, dtype=np.float64)
    corr[:128] = float(N * N) * b3e[:128].astype(np.float64)
    corr[128:] = float(2 * CH) * b3e[128:].astype(np.float64)
    fb1e = (fb1e.astype(np.float64) + F1e.astype(np.float64) @ corr).astype(np.float32)

    shared = {
        "w0l": np.ascontiguousarray(W0e[:, :D].T),
        "w0r": np.ascontiguousarray(W0e[:, D:].T),
        "b1": cols2(b1e),
        "w2": tile4(W2e.T), "b2": cols2(b2e),
        "w3": tile4(W3e.T),
        "b3": np.stack([-b3e[:128], b3e[128:], -b3e[128:]],
                       axis=1).astype(np.float32),
        "fw1": tile4(F1e.T), "fb1": cols2(fb1e),
        "fw2": tile4(F2e.T), "fb2": cols2(fb2e),
        "fow": np.ascontiguousarray(
            inputs["fo_W"].T.reshape(2, 128, OUT)).astype(np.float32),
        "fob": np.tile(inputs["fo_b"].reshape(1, OUT), (BC, 1)).astype(np.float32),
    }
    shared["w0l"] = shared["w0l"].astype(np.float16)
    shared["w0r"] = shared["w0r"].astype(np.float16)
    x = inputs["x"]
    in_maps = []
    for c in range(N_CORES):
        xc = x[c * BC:(c + 1) * BC]  # [BC, N, D]
        xt = np.ascontiguousarray(xc.transpose(2, 0, 1).reshape(D, BC * N))
        in_maps.append({"xt": xt.astype(np.float16), **shared})
    return in_maps


def kernel(**inputs):
    global LAST
    inputs = {k: np.asarray(v, dtype=np.float32) for k, v in inputs.items()}
    if "nc" not in _CACHE:
        _CACHE["nc"] = _build()
    nc = _CACHE["nc"]
    in_maps = _prep(inputs)
    res = run_bass_kernel_spmd(nc, in_maps, core_ids=list(range(N_CORES)),
                               trace=TRACE)
    LAST = res
    out = np.concatenate([res.results[c]["out"] for c in range(N_CORES)], axis=0)
    return out.astype(np.float32)
